# revision 33
# baseline (speedup 1.0000x reference)
"""DbrxExperts MoE kernel for 8 Trainium2 NeuronCores (expert-parallel).

Problem: E=16 experts, top_k=4, H=2048, F=4096, T=64 tokens.
out = sum_e r[:, e] * (silu(x @ w1_e.T) * (x @ v1_e.T)) @ w2_e
with r = scatter-add of top_weights into dense [T, E].

Strategy: expert-parallel across 8 cores (2 experts per core), with the
two experts PAIRED across the PE array's column halves so both stream
concurrently (M=64 alone wastes half the 128-wide array):
  - gate/up matmuls: expert 0 writes PSUM partitions 0-63, expert 1
    writes partitions 64-127; consecutive matmuls alternate column
    groups, so the PE executes two M=64 matmuls at once.
  - w1/v1 are stored fp8e3 (e3m4) with a global power-of-2 scale folded
    exactly into the bf16 x operand planes; w2 is fp8e3 for the second
    half of the ffn dim (scale folded into the hT drain as an exact pow2
    tensor_scalar_mul) and bf16 for the first half. Measured rel-err
    1.90e-2 < 2e-2.
  - h [128(t: e0|e1), width] is transposed in full 128x128 blocks; the
    transposed tile has each expert's tokens in separate 64-col slices,
    used directly as the down-projection stationary operands.
  - the ffn dim is processed in TAPERED chunks (4,4,4,4,4,4,4,3,1
    blocks of 128): large chunks give big efficient DMAs mid-stream,
    and the single small trailing chunk shrinks the PE pipeline-drain
    tail after the last weight bytes land (more than one small chunk
    stalls the stream on tile-pool buffer recycling - measured).

Weight layouts are pre-swizzled on the host so each chunk's weights are
fully contiguous DMA transfers.
"""

import os
import sys
import types

import numpy as np
import ml_dtypes

BF16 = ml_dtypes.bfloat16
F8E3 = ml_dtypes.float8_e3m4

E, TOPK, H, F = 16, 4, 2048, 4096
T = 64
N_CORES = 8
EPC = E // N_CORES          # experts per core = 2
KT = H // 128               # 16 k-tiles of 128 over H
NB = F // 128               # 32 f-blocks of 128 over F
PLAN = [4, 4, 4, 4, 4, 4, 4, 3, 1]   # blocks per chunk (sum = 32)
CUM = [sum(PLAN[:i]) for i in range(len(PLAN))]
NCH = len(PLAN)
NBF_BLOCKS = 16             # leading f-blocks with bf16 w2; rest fp8
F8MAX = 15.5
assert sum(PLAN) == NB and NBF_BLOCKS in CUM
# dram free widths padded to pow2 per-partition strides (HBM channel balance)
W2S_W = 1 << (EPC * NBF_BLOCKS * H - 1).bit_length()
W2Q_W = 1 << (EPC * (NB - NBF_BLOCKS) * H - 1).bit_length()


def _ensure_axon_hooks():
    """antenv.axon_hooks is missing from the stub antenv shipped in some
    containers; run_bass_kernel_spmd(trace=True) imports it under axon.
    Register the ctypes NTFF hook when libaxon_pjrt.so is present, else a
    None-returning stub so tracing degrades instead of crashing."""
    try:
        import antenv.axon_hooks  # noqa: F401
        return
    except ImportError:
        pass
    try:
        import antenv
    except ImportError:
        return
    mod = types.ModuleType("antenv.axon_hooks")
    _hook = [None]
    mod.set_axon_ntff_profile_hook = lambda h: _hook.__setitem__(0, h)
    mod.get_axon_ntff_profile_hook = lambda: _hook[0]
    sys.modules["antenv.axon_hooks"] = mod
    antenv.axon_hooks = mod
    try:
        from trn_agent_boot.trn_boot import _ntff_profile_via_ctypes

        so_path = "/opt/axon/libaxon_pjrt.so"
        if os.path.exists(so_path):
            h = _ntff_profile_via_ctypes(so_path)
            if h is not None:
                mod.set_axon_ntff_profile_hook(h)
    except Exception:
        pass


def _build_nc(k2):
    import concourse.mybir as mybir
    import concourse.tile as tile
    from concourse import bacc
    from concourse.masks import make_identity

    f32 = mybir.dt.float32
    bf16 = mybir.dt.bfloat16
    f8 = mybir.dt.float8e3

    nc = bacc.Bacc("TRN2", debug=False, num_devices=N_CORES)
    xt_d = nc.dram_tensor("xt", [1 + EPC, 128, KT * T], bf16, kind="ExternalInput")
    w1_d = nc.dram_tensor("w1t", [128, EPC * KT * NB * 128], f8, kind="ExternalInput")
    v1_d = nc.dram_tensor("v1t", [128, EPC * KT * NB * 128], f8, kind="ExternalInput")
    w2_d = nc.dram_tensor("w2s", [128, W2S_W], bf16, kind="ExternalInput")
    w2q_d = nc.dram_tensor("w2q", [128, W2Q_W], f8, kind="ExternalInput")
    out_d = nc.dram_tensor("out", [T, H], f32, kind="ExternalOutput")

    act = mybir.ActivationFunctionType
    k2inv = float(2.0**-k2)

    with tile.TileContext(nc) as tc:
        with (
            tc.tile_pool(name="const", bufs=1) as const_pool,
            tc.tile_pool(name="w1", bufs=3) as w1_pool,
            tc.tile_pool(name="v1", bufs=3) as v1_pool,
            tc.tile_pool(name="w2", bufs=2) as w2_pool,
            tc.tile_pool(name="acts", bufs=4) as acts_pool,
            tc.tile_pool(name="ps_gate", bufs=2, space="PSUM") as ps_gate,
            tc.tile_pool(name="ps_up", bufs=2, space="PSUM") as ps_up,
            tc.tile_pool(name="ps_tp", bufs=2, space="PSUM") as ps_tp,
            tc.tile_pool(name="ps_down", bufs=1, space="PSUM") as ps_down,
        ):
            xg_sb = const_pool.tile([128, KT * T], bf16)
            nc.scalar.dma_start(xg_sb[:], xt_d[0])
            xu_sb = []
            for e in range(EPC):
                t_ = const_pool.tile([128, KT * T], bf16, tag=f"xu{e}")
                nc.scalar.dma_start(t_[:], xt_d[1 + e])
                xu_sb.append(t_)
            ident = const_pool.tile([128, 128], bf16)
            make_identity(nc, ident)

            wtiles = {}

            def issue_wv(c):
                nb = PLAN[c]
                width = EPC * KT * nb * 128
                base = EPC * KT * 128 * CUM[c]
                w1c = w1_pool.tile([128, width], f8, tag="w1c")
                v1c = v1_pool.tile([128, width], f8, tag="v1c")
                nc.sync.dma_start(w1c[:], w1_d[:, base : base + width])
                nc.scalar.dma_start(v1c[:], v1_d[:, base : base + width])
                wtiles[c] = (w1c, v1c)

            def issue_w2(c):
                nb = PLAN[c]
                fp8w2 = CUM[c] >= NBF_BLOCKS
                width = EPC * nb * H
                half = nb * H
                if fp8w2:
                    src, base = w2q_d, EPC * H * (CUM[c] - NBF_BLOCKS)
                    w2c = w2_pool.tile([128, width], f8, tag="w2c")
                else:
                    src, base = w2_d, EPC * H * CUM[c]
                    w2c = w2_pool.tile([128, width], bf16, tag="w2c")
                nc.sync.dma_start(w2c[:, 0:half], src[:, base : base + half])
                nc.scalar.dma_start(
                    w2c[:, half:width], src[:, base + half : base + width]
                )
                return w2c, fp8w2

            # persistent down-projection accumulator:
            # [0:64, :] = hid 0..1023, [64:128, :] = hid 1024..2047
            down_ps = ps_down.tile([128, 1024], mybir.dt.float32)

            issue_wv(0)
            for c in range(NCH):
                nb = PLAN[c]
                fw = nb * 128
                if c == NCH - 2:
                    # penultimate chunk: its w2 goes BEFORE the last chunk's
                    # w1/v1 on the rings so its down-matmuls overlap the
                    # stream tail instead of serializing after it
                    w2c, fp8w2 = issue_w2(c)
                    issue_wv(c + 1)
                elif c + 1 < NCH:
                    # next chunk's w1/v1 ahead of this chunk's w2 so the
                    # trailing gate/up chains start before the stream ends
                    issue_wv(c + 1)
                    w2c, fp8w2 = issue_w2(c)
                else:
                    w2c, fp8w2 = issue_w2(c)
                w1c, v1c = wtiles.pop(c)

                gate_ps = ps_gate.tile([128, fw], mybir.dt.float32, tag="gate")
                up_ps = ps_up.tile([128, fw], mybir.dt.float32, tag="up")
                # both experts share the xg stationary; alternate column
                # groups (out partitions 0-63 / 64-127) so the two M=64
                # matmuls run concurrently in the PE array.
                for i in range(KT):
                    for e in range(EPC):
                        lo = (e * KT + i) * fw
                        nc.tensor.matmul(
                            gate_ps[64 * e : 64 * e + T, :],
                            xg_sb[:, i * T : (i + 1) * T],
                            w1c[:, lo : lo + fw],
                            start=(i == 0),
                            stop=(i == KT - 1),
                        )
                for i in range(KT):
                    for e in range(EPC):
                        lo = (e * KT + i) * fw
                        nc.tensor.matmul(
                            up_ps[64 * e : 64 * e + T, :],
                            xu_sb[e][:, i * T : (i + 1) * T],
                            v1c[:, lo : lo + fw],
                            start=(i == 0),
                            stop=(i == KT - 1),
                        )

                gate_s = acts_pool.tile([128, fw], bf16, tag="gate_s")
                nc.scalar.activation(gate_s[:], gate_ps[:], act.Silu)
                h = acts_pool.tile([128, fw], bf16, tag="h")
                nc.vector.tensor_mul(h[:], gate_s[:], up_ps[:])

                # transpose h in full 128x128 blocks: block j becomes
                # [128 f, 128 t] with e0 tokens in cols 0-63, e1 in 64-127
                tp_ps = ps_tp.tile([128, fw], bf16, tag="tp")
                for j in range(nb):
                    nc.tensor.transpose(
                        tp_ps[:, j * 128 : (j + 1) * 128],
                        h[:, j * 128 : (j + 1) * 128],
                        ident[:],
                    )
                hT = acts_pool.tile([128, fw], bf16, tag="hT")
                if fp8w2:
                    # fold the w2 fp8 scale in here: exact pow2 shift on bf16
                    nc.vector.tensor_scalar_mul(hT[:], tp_ps[:], k2inv)
                else:
                    nc.vector.tensor_copy(hT[:], tp_ps[:])

                for e in range(EPC):
                    for j in range(nb):
                        st = hT[:, j * 128 + 64 * e : j * 128 + 64 * e + T]
                        wb = (e * nb + j) * H
                        first = c == 0 and e == 0 and j == 0
                        last = c == NCH - 1 and e == EPC - 1 and j == nb - 1
                        # hid quarters q0..q3; order (0,2,1,3) alternates
                        # column groups for PE concurrency
                        for q in (0, 2, 1, 3):
                            if q < 2:
                                dst = down_ps[0:T, q * 512 : (q + 1) * 512]
                            else:
                                dst = down_ps[
                                    64 : 64 + T, (q - 2) * 512 : (q - 1) * 512
                                ]
                            nc.tensor.matmul(
                                dst,
                                st,
                                w2c[:, wb + q * 512 : wb + (q + 1) * 512],
                                start=first,
                                stop=last,
                            )

            # final drain in quarters; concurrent DVE/ACT ops are paired on
            # DIFFERENT PSUM banks (free 0:512 = bank0, 512:1024 = bank1),
            # since V+S only run parallel on distinct banks
            out_sb = const_pool.tile([128, 1024], mybir.dt.float32)
            nc.vector.tensor_copy(out_sb[0:T, 0:512], down_ps[0:T, 0:512])
            nc.sync.dma_start(out_d[:, 0:512], out_sb[0:T, 0:512])
            nc.scalar.activation(
                out_sb[0:T, 512:1024], down_ps[0:T, 512:1024], act.Copy
            )
            nc.scalar.dma_start(out_d[:, 512:1024], out_sb[0:T, 512:1024])
            nc.vector.tensor_copy(
                out_sb[64 : 64 + T, 0:512], down_ps[64 : 64 + T, 0:512]
            )
            nc.sync.dma_start(out_d[:, 1024:1536], out_sb[64 : 64 + T, 0:512])
            nc.scalar.activation(
                out_sb[64 : 64 + T, 512:1024],
                down_ps[64 : 64 + T, 512:1024],
                act.Copy,
            )
            nc.scalar.dma_start(out_d[:, 1536:2048], out_sb[64 : 64 + T, 512:1024])

    nc.compile()
    return nc


_NC_CACHE = {}


def _get_nc(k2):
    if k2 not in _NC_CACHE:
        _NC_CACHE[k2] = _build_nc(k2)
    return _NC_CACHE[k2]


def _swz_ffn_chunks(wt):
    """[H, F] (h, f) -> list of per-chunk [128, KT*nb*128] arrays with
    [p, (i*nb + jrel)*128 + f'] = wt[i*128 + p, (CUM[c] + jrel)*128 + f']."""
    a = wt.reshape(KT, 128, NB, 128)          # (i, p, jb, f')
    out = []
    for c in range(NCH):
        nb = PLAN[c]
        s = a[:, :, CUM[c] : CUM[c] + nb, :]  # (i, p, nb, f')
        out.append(
            np.ascontiguousarray(s.transpose(1, 0, 2, 3)).reshape(128, KT * nb * 128)
        )
    return out


def _swz_down_chunks(w2e):
    """[F, H] (f, hid) -> list of per-chunk [128, nb*H] arrays with
    [p, jrel*H + hid] = w2e[(CUM[c] + jrel)*128 + p, hid]."""
    out = []
    for c in range(NCH):
        nb = PLAN[c]
        s = w2e[CUM[c] * 128 : (CUM[c] + nb) * 128].reshape(nb, 128, H)
        out.append(np.ascontiguousarray(s.transpose(1, 0, 2)).reshape(128, nb * H))
    return out


def _interleave(per_expert_chunks, sel):
    """per_expert_chunks: [e] -> list over chunks; concat (chunk, expert)
    order along the free axis for the chunk ids in sel."""
    parts = []
    for c in sel:
        for pe in per_expert_chunks:
            parts.append(pe[c])
    return np.ascontiguousarray(np.concatenate(parts, axis=1))


def _pow2_scale(amax):
    return int(np.floor(np.log2(F8MAX * 0.98 / amax)))


def kernel(x, weights, top_weights, top_experts, w1, v1, w2):
    _ensure_axon_hooks()
    from concourse.bass_utils import run_bass_kernel_spmd

    x = np.asarray(x, dtype=np.float32).reshape(T, H)
    top_weights = np.asarray(top_weights, dtype=np.float32)
    top_experts = np.asarray(top_experts).astype(np.int64)
    w1 = np.asarray(w1, dtype=np.float32).reshape(E, F, H)
    v1 = np.asarray(v1, dtype=np.float32).reshape(E, F, H)
    w2 = np.asarray(w2, dtype=np.float32).reshape(E, F, H)

    # dense routing weights [T, E] (scatter-ADD: duplicate experts sum)
    r = np.zeros((T, E), np.float32)
    np.add.at(r, (np.arange(T)[:, None], top_experts), top_weights)

    # global power-of-2 scales for the fp8 weights; w1/v1 scales fold
    # exactly into the bf16 x operand planes, w2's into the hT drain
    k1 = _pow2_scale(np.abs(w1).max())
    kv = _pow2_scale(np.abs(v1).max())
    k2 = _pow2_scale(np.abs(w2).max())
    w1q = (w1 * np.float32(2.0**k1)).astype(F8E3)
    v1q = (v1 * np.float32(2.0**kv)).astype(F8E3)
    w2q = (w2 * np.float32(2.0**k2)).astype(F8E3)

    # x transposed/swizzled: [128, KT*T] with [p, i*T + t] = a[t, i*128 + p]
    def swz_x(a):
        return np.ascontiguousarray(
            a.T.reshape(KT, 128, T).transpose(1, 0, 2)
        ).reshape(128, KT * T).astype(BF16)

    xg = swz_x(x * np.float32(2.0**-k1))

    bf_sel = [c for c in range(NCH) if CUM[c] < NBF_BLOCKS]
    q_sel = [c for c in range(NCH) if CUM[c] >= NBF_BLOCKS]

    in_maps = []
    for core in range(N_CORES):
        es = [core * EPC + k for k in range(EPC)]
        w1t = _interleave([_swz_ffn_chunks(w1q[e].T) for e in es], range(NCH))
        v1t = _interleave([_swz_ffn_chunks(v1q[e].T) for e in es], range(NCH))
        w2bf = _interleave(
            [_swz_down_chunks(w2[e].astype(BF16)) for e in es], bf_sel
        )
        w2qs = _interleave([_swz_down_chunks(w2q[e]) for e in es], q_sel)
        w2bf_p = np.zeros((128, W2S_W), BF16)
        w2bf_p[:, : w2bf.shape[1]] = w2bf
        w2qs_p = np.zeros((128, W2Q_W), F8E3)
        w2qs_p[:, : w2qs.shape[1]] = w2qs
        w2bf, w2qs = w2bf_p, w2qs_p
        # plane 0: x*2^-k1 for the gate path; planes 1+k: r_e-scaled x
        # (times 2^-kv) for the up path
        xt_planes = np.stack(
            [xg]
            + [swz_x(x * (r[:, ee : ee + 1] * np.float32(2.0**-kv))) for ee in es],
            axis=0,
        )
        in_maps.append(
            {
                "xt": xt_planes,
                "w1t": w1t,
                "v1t": v1t,
                "w2s": w2bf,
                "w2q": w2qs,
            }
        )

    nc = _get_nc(k2)
    res = run_bass_kernel_spmd(nc, in_maps, core_ids=list(range(N_CORES)))
    out = np.zeros((T, H), np.float32)
    for c in range(N_CORES):
        out += res.results[c]["out"]
    return out.reshape(64, 1, H)


# revision 35
# speedup vs baseline: 1.1670x; 1.1670x over previous
"""DbrxExperts MoE kernel for 8 Trainium2 NeuronCores (expert-parallel).

Problem: E=16 experts, top_k=4, H=2048, F=4096, T=64 tokens.
out = sum_e r[:, e] * (silu(x @ w1_e.T) * (x @ v1_e.T)) @ w2_e
with r = scatter-add of top_weights into dense [T, E].

Strategy: expert-parallel across 8 cores (2 experts per core), with the
two experts PAIRED across the PE array's column halves so both stream
concurrently (M=64 alone wastes half the 128-wide array):
  - gate/up matmuls: expert 0 writes PSUM partitions 0-63, expert 1
    writes partitions 64-127; consecutive matmuls alternate column
    groups, so the PE executes two M=64 matmuls at once.
  - w1/v1 are stored fp8e3 (e3m4) with a global power-of-2 scale folded
    exactly into the bf16 x operand planes; w2 is fp8e3 for the second
    half of the ffn dim (scale folded into the hT drain as an exact pow2
    tensor_scalar_mul) and bf16 for the first half. Measured rel-err
    1.90e-2 < 2e-2.
  - h [128(t: e0|e1), width] is transposed in full 128x128 blocks; the
    transposed tile has each expert's tokens in separate 64-col slices,
    used directly as the down-projection stationary operands.
  - the ffn dim is processed in TAPERED chunks (4,4,4,4,4,4,4,3,1
    blocks of 128): large chunks give big efficient DMAs mid-stream,
    and the single small trailing chunk shrinks the PE pipeline-drain
    tail after the last weight bytes land (more than one small chunk
    stalls the stream on tile-pool buffer recycling - measured).

Weight layouts are pre-swizzled on the host so each chunk's weights are
fully contiguous DMA transfers.
"""

import os
import sys
import types

import numpy as np
import ml_dtypes

BF16 = ml_dtypes.bfloat16
F8E3 = ml_dtypes.float8_e3m4

E, TOPK, H, F = 16, 4, 2048, 4096
T = 64
N_CORES = 8
EPC = E // N_CORES          # experts per core = 2
KT = H // 128               # 16 k-tiles of 128 over H
NB = F // 128               # 32 f-blocks of 128 over F
PLAN = [4, 4, 4, 4, 4, 4, 4, 3, 1]   # blocks per chunk (sum = 32)
CUM = [sum(PLAN[:i]) for i in range(len(PLAN))]
NCH = len(PLAN)
NBF_BLOCKS = 16             # leading f-blocks with bf16 w2; rest fp8
F8MAX = 15.5
assert sum(PLAN) == NB and NBF_BLOCKS in CUM
# dram free widths padded to pow2 per-partition strides (HBM channel balance)
W2S_W = 1 << (EPC * NBF_BLOCKS * H - 1).bit_length()
W2Q_W = 1 << (EPC * (NB - NBF_BLOCKS) * H - 1).bit_length()


def _ensure_axon_hooks():
    """antenv.axon_hooks is missing from the stub antenv shipped in some
    containers; run_bass_kernel_spmd(trace=True) imports it under axon.
    Register the ctypes NTFF hook when libaxon_pjrt.so is present, else a
    None-returning stub so tracing degrades instead of crashing."""
    try:
        import antenv.axon_hooks  # noqa: F401
        return
    except ImportError:
        pass
    try:
        import antenv
    except ImportError:
        return
    mod = types.ModuleType("antenv.axon_hooks")
    _hook = [None]
    mod.set_axon_ntff_profile_hook = lambda h: _hook.__setitem__(0, h)
    mod.get_axon_ntff_profile_hook = lambda: _hook[0]
    sys.modules["antenv.axon_hooks"] = mod
    antenv.axon_hooks = mod
    try:
        from trn_agent_boot.trn_boot import _ntff_profile_via_ctypes

        so_path = "/opt/axon/libaxon_pjrt.so"
        if os.path.exists(so_path):
            h = _ntff_profile_via_ctypes(so_path)
            if h is not None:
                mod.set_axon_ntff_profile_hook(h)
    except Exception:
        pass


def _build_nc(k2):
    import concourse.mybir as mybir
    import concourse.tile as tile
    from concourse import bacc
    from concourse.masks import make_identity

    f32 = mybir.dt.float32
    bf16 = mybir.dt.bfloat16
    f8 = mybir.dt.float8e3

    nc = bacc.Bacc("TRN2", debug=False, num_devices=N_CORES)
    xt_d = nc.dram_tensor("xt", [1 + EPC, 128, KT * T], bf16, kind="ExternalInput")
    w1_d = nc.dram_tensor("w1t", [128, EPC * KT * NB * 128], f8, kind="ExternalInput")
    v1_d = nc.dram_tensor("v1t", [128, EPC * KT * NB * 128], f8, kind="ExternalInput")
    w2_d = nc.dram_tensor("w2s", [128, W2S_W], bf16, kind="ExternalInput")
    w2q_d = nc.dram_tensor("w2q", [128, W2Q_W], f8, kind="ExternalInput")
    out_d = nc.dram_tensor("out", [T, H], f32, kind="ExternalOutput")

    act = mybir.ActivationFunctionType
    k2inv = float(2.0**-k2)

    with tile.TileContext(nc) as tc:
        with (
            tc.tile_pool(name="const", bufs=1) as const_pool,
            tc.tile_pool(name="w1", bufs=3) as w1_pool,
            tc.tile_pool(name="v1", bufs=3) as v1_pool,
            tc.tile_pool(name="w2", bufs=2) as w2_pool,
            tc.tile_pool(name="acts", bufs=4) as acts_pool,
            tc.tile_pool(name="ps_gate", bufs=2, space="PSUM") as ps_gate,
            tc.tile_pool(name="ps_up", bufs=2, space="PSUM") as ps_up,
            tc.tile_pool(name="ps_tp", bufs=2, space="PSUM") as ps_tp,
            tc.tile_pool(name="ps_down", bufs=1, space="PSUM") as ps_down,
        ):
            xg_sb = const_pool.tile([128, KT * T], bf16)
            nc.scalar.dma_start(xg_sb[:], xt_d[0])
            xu_sb = []
            for e in range(EPC):
                t_ = const_pool.tile([128, KT * T], bf16, tag=f"xu{e}")
                nc.scalar.dma_start(t_[:], xt_d[1 + e])
                xu_sb.append(t_)
            ident = const_pool.tile([128, 128], bf16)
            make_identity(nc, ident)

            wtiles = {}

            def issue_wv(c):
                nb = PLAN[c]
                width = EPC * KT * nb * 128
                base = EPC * KT * 128 * CUM[c]
                w1c = w1_pool.tile([128, width], f8, tag="w1c")
                v1c = v1_pool.tile([128, width], f8, tag="v1c")
                nc.sync.dma_start(w1c[:], w1_d[:, base : base + width])
                nc.scalar.dma_start(v1c[:], v1_d[:, base : base + width])
                wtiles[c] = (w1c, v1c)

            def issue_w2(c):
                nb = PLAN[c]
                fp8w2 = CUM[c] >= NBF_BLOCKS
                width = EPC * nb * H
                half = nb * H
                if fp8w2:
                    src, base = w2q_d, EPC * H * (CUM[c] - NBF_BLOCKS)
                    w2c = w2_pool.tile([128, width], f8, tag="w2c")
                else:
                    src, base = w2_d, EPC * H * CUM[c]
                    w2c = w2_pool.tile([128, width], bf16, tag="w2c")
                nc.sync.dma_start(w2c[:, 0:half], src[:, base : base + half])
                nc.scalar.dma_start(
                    w2c[:, half:width], src[:, base + half : base + width]
                )
                return w2c, fp8w2

            # persistent down-projection accumulator:
            # [0:64, :] = hid 0..1023, [64:128, :] = hid 1024..2047
            down_ps = ps_down.tile([128, 1024], mybir.dt.float32)

            issue_wv(0)
            for c in range(NCH):
                nb = PLAN[c]
                fw = nb * 128
                if c + 1 < NCH:
                    # next chunk's w1/v1 ahead of this chunk's w2 so the
                    # trailing gate/up chains start before the stream ends
                    issue_wv(c + 1)
                w2c, fp8w2 = issue_w2(c)
                w1c, v1c = wtiles.pop(c)

                gate_ps = ps_gate.tile([128, fw], mybir.dt.float32, tag="gate")
                up_ps = ps_up.tile([128, fw], mybir.dt.float32, tag="up")
                # both experts share the xg stationary; alternate column
                # groups (out partitions 0-63 / 64-127) so the two M=64
                # matmuls run concurrently in the PE array.
                for i in range(KT):
                    for e in range(EPC):
                        lo = (e * KT + i) * fw
                        nc.tensor.matmul(
                            gate_ps[64 * e : 64 * e + T, :],
                            xg_sb[:, i * T : (i + 1) * T],
                            w1c[:, lo : lo + fw],
                            start=(i == 0),
                            stop=(i == KT - 1),
                        )
                for i in range(KT):
                    for e in range(EPC):
                        lo = (e * KT + i) * fw
                        nc.tensor.matmul(
                            up_ps[64 * e : 64 * e + T, :],
                            xu_sb[e][:, i * T : (i + 1) * T],
                            v1c[:, lo : lo + fw],
                            start=(i == 0),
                            stop=(i == KT - 1),
                        )

                gate_s = acts_pool.tile([128, fw], bf16, tag="gate_s")
                nc.scalar.activation(gate_s[:], gate_ps[:], act.Silu)
                h = acts_pool.tile([128, fw], bf16, tag="h")
                nc.vector.tensor_mul(h[:], gate_s[:], up_ps[:])

                # transpose h in full 128x128 blocks: block j becomes
                # [128 f, 128 t] with e0 tokens in cols 0-63, e1 in 64-127
                tp_ps = ps_tp.tile([128, fw], bf16, tag="tp")
                for j in range(nb):
                    nc.tensor.transpose(
                        tp_ps[:, j * 128 : (j + 1) * 128],
                        h[:, j * 128 : (j + 1) * 128],
                        ident[:],
                    )
                hT = acts_pool.tile([128, fw], bf16, tag="hT")
                if fp8w2:
                    # fold the w2 fp8 scale in here: exact pow2 shift on bf16
                    nc.vector.tensor_scalar_mul(hT[:], tp_ps[:], k2inv)
                else:
                    nc.vector.tensor_copy(hT[:], tp_ps[:])

                for e in range(EPC):
                    for j in range(nb):
                        st = hT[:, j * 128 + 64 * e : j * 128 + 64 * e + T]
                        wb = (e * nb + j) * H
                        first = c == 0 and e == 0 and j == 0
                        last = c == NCH - 1 and e == EPC - 1 and j == nb - 1
                        # hid quarters q0..q3; order (0,2,1,3) alternates
                        # column groups for PE concurrency
                        for q in (0, 2, 1, 3):
                            if q < 2:
                                dst = down_ps[0:T, q * 512 : (q + 1) * 512]
                            else:
                                dst = down_ps[
                                    64 : 64 + T, (q - 2) * 512 : (q - 1) * 512
                                ]
                            nc.tensor.matmul(
                                dst,
                                st,
                                w2c[:, wb + q * 512 : wb + (q + 1) * 512],
                                start=first,
                                stop=last,
                            )

            # final drain in quarters, alternating engines and rings, so
            # each region streams out as soon as its accumulation stops
            out_sb = const_pool.tile([128, 1024], mybir.dt.float32)
            nc.vector.tensor_copy(out_sb[0:T, 0:512], down_ps[0:T, 0:512])
            nc.sync.dma_start(out_d[:, 0:512], out_sb[0:T, 0:512])
            nc.scalar.activation(
                out_sb[64 : 64 + T, 0:512], down_ps[64 : 64 + T, 0:512], act.Copy
            )
            nc.scalar.dma_start(out_d[:, 1024:1536], out_sb[64 : 64 + T, 0:512])
            nc.vector.tensor_copy(out_sb[0:T, 512:1024], down_ps[0:T, 512:1024])
            nc.sync.dma_start(out_d[:, 512:1024], out_sb[0:T, 512:1024])
            nc.scalar.activation(
                out_sb[64 : 64 + T, 512:1024],
                down_ps[64 : 64 + T, 512:1024],
                act.Copy,
            )
            nc.scalar.dma_start(out_d[:, 1536:2048], out_sb[64 : 64 + T, 512:1024])

    nc.compile()
    return nc


_NC_CACHE = {}


def _get_nc(k2):
    if k2 not in _NC_CACHE:
        _NC_CACHE[k2] = _build_nc(k2)
    return _NC_CACHE[k2]


def _swz_ffn_chunks(wt):
    """[H, F] (h, f) -> list of per-chunk [128, KT*nb*128] arrays with
    [p, (i*nb + jrel)*128 + f'] = wt[i*128 + p, (CUM[c] + jrel)*128 + f']."""
    a = wt.reshape(KT, 128, NB, 128)          # (i, p, jb, f')
    out = []
    for c in range(NCH):
        nb = PLAN[c]
        s = a[:, :, CUM[c] : CUM[c] + nb, :]  # (i, p, nb, f')
        out.append(
            np.ascontiguousarray(s.transpose(1, 0, 2, 3)).reshape(128, KT * nb * 128)
        )
    return out


def _swz_down_chunks(w2e):
    """[F, H] (f, hid) -> list of per-chunk [128, nb*H] arrays with
    [p, jrel*H + hid] = w2e[(CUM[c] + jrel)*128 + p, hid]."""
    out = []
    for c in range(NCH):
        nb = PLAN[c]
        s = w2e[CUM[c] * 128 : (CUM[c] + nb) * 128].reshape(nb, 128, H)
        out.append(np.ascontiguousarray(s.transpose(1, 0, 2)).reshape(128, nb * H))
    return out


def _interleave(per_expert_chunks, sel):
    """per_expert_chunks: [e] -> list over chunks; concat (chunk, expert)
    order along the free axis for the chunk ids in sel."""
    parts = []
    for c in sel:
        for pe in per_expert_chunks:
            parts.append(pe[c])
    return np.ascontiguousarray(np.concatenate(parts, axis=1))


def _pow2_scale(amax):
    return int(np.floor(np.log2(F8MAX * 0.98 / amax)))


def kernel(x, weights, top_weights, top_experts, w1, v1, w2):
    _ensure_axon_hooks()
    from concourse.bass_utils import run_bass_kernel_spmd

    x = np.asarray(x, dtype=np.float32).reshape(T, H)
    top_weights = np.asarray(top_weights, dtype=np.float32)
    top_experts = np.asarray(top_experts).astype(np.int64)
    w1 = np.asarray(w1, dtype=np.float32).reshape(E, F, H)
    v1 = np.asarray(v1, dtype=np.float32).reshape(E, F, H)
    w2 = np.asarray(w2, dtype=np.float32).reshape(E, F, H)

    # dense routing weights [T, E] (scatter-ADD: duplicate experts sum)
    r = np.zeros((T, E), np.float32)
    np.add.at(r, (np.arange(T)[:, None], top_experts), top_weights)

    # global power-of-2 scales for the fp8 weights; w1/v1 scales fold
    # exactly into the bf16 x operand planes, w2's into the hT drain
    k1 = _pow2_scale(np.abs(w1).max())
    kv = _pow2_scale(np.abs(v1).max())
    k2 = _pow2_scale(np.abs(w2).max())
    w1q = (w1 * np.float32(2.0**k1)).astype(F8E3)
    v1q = (v1 * np.float32(2.0**kv)).astype(F8E3)
    w2q = (w2 * np.float32(2.0**k2)).astype(F8E3)

    # x transposed/swizzled: [128, KT*T] with [p, i*T + t] = a[t, i*128 + p]
    def swz_x(a):
        return np.ascontiguousarray(
            a.T.reshape(KT, 128, T).transpose(1, 0, 2)
        ).reshape(128, KT * T).astype(BF16)

    xg = swz_x(x * np.float32(2.0**-k1))

    bf_sel = [c for c in range(NCH) if CUM[c] < NBF_BLOCKS]
    q_sel = [c for c in range(NCH) if CUM[c] >= NBF_BLOCKS]

    in_maps = []
    for core in range(N_CORES):
        es = [core * EPC + k for k in range(EPC)]
        w1t = _interleave([_swz_ffn_chunks(w1q[e].T) for e in es], range(NCH))
        v1t = _interleave([_swz_ffn_chunks(v1q[e].T) for e in es], range(NCH))
        w2bf = _interleave(
            [_swz_down_chunks(w2[e].astype(BF16)) for e in es], bf_sel
        )
        w2qs = _interleave([_swz_down_chunks(w2q[e]) for e in es], q_sel)
        w2bf_p = np.zeros((128, W2S_W), BF16)
        w2bf_p[:, : w2bf.shape[1]] = w2bf
        w2qs_p = np.zeros((128, W2Q_W), F8E3)
        w2qs_p[:, : w2qs.shape[1]] = w2qs
        w2bf, w2qs = w2bf_p, w2qs_p
        # plane 0: x*2^-k1 for the gate path; planes 1+k: r_e-scaled x
        # (times 2^-kv) for the up path
        xt_planes = np.stack(
            [xg]
            + [swz_x(x * (r[:, ee : ee + 1] * np.float32(2.0**-kv))) for ee in es],
            axis=0,
        )
        in_maps.append(
            {
                "xt": xt_planes,
                "w1t": w1t,
                "v1t": v1t,
                "w2s": w2bf,
                "w2q": w2qs,
            }
        )

    nc = _get_nc(k2)
    res = run_bass_kernel_spmd(nc, in_maps, core_ids=list(range(N_CORES)))
    out = np.zeros((T, H), np.float32)
    for c in range(N_CORES):
        out += res.results[c]["out"]
    return out.reshape(64, 1, H)
